# revision 5
# baseline (speedup 1.0000x reference)
"""Exponentiated-quadratic (RBF) kernel matrix on 8 Trainium2 NeuronCores.

K[i, j] = sigma * exp(-0.5 * ||x1_i/rho - x2_j/rho||^2)
        with sigma = exp(log_sigma)^2, rho = exp(log_rho)

Strategy (v2)
-------------
Row-shard x1 across the 8 cores (512 rows each), replicate x2.

Matmul: the entire contraction for one 512-column PSUM bank is ONE K=99
stacked matmul — lhsT rows = [Ah; Ah; Al; ones3], rhs rows =
[Bh; Bl; Bh; yn3] — since PE cost is free-dim bound (~518 cyc @2.4GHz per
512 cols), not K bound.  That is 4x fewer PE cycles than running the bf16
splits as separate passes (full 3-pass precision kept: Ah.Bh+Ah.Bl+Al.Bh,
plus the -0.5||y||^2 row via 3 bf16 splits).

Epilogue: one ScalarE exp-activation per [128,2048] PSUM tile with the
per-row bias (-0.5||x_i||^2 + 2 log_sigma) applied via ACT's free
per-partition bias, writing bf16 (the |err| <= 2^-9*K of bf16 is far
inside the 2e-2 rel-to-scale gate; host upcasts to fp32).  bf16 halves
the output-store HBM traffic.  A tiny warmup ACT at kernel start pulls
the ~1.3us exp table load off the critical path.

DMA: 3 input loads (A+bias fused [128,520]; B cols 0:2048; B cols
2048:4096) + 5 output stores = 8 total (HWDGE lane budget).  Stores are
issued per row-block as soon as its second activation lands; the last
block stores in halves right after each activation.

walrus in this container rejects instructions carrying more than one
semaphore wait: engine-queue nops "observe" DMA semaphores ahead of the
consumers, activations carry only their PE wait (ACT->ACT PSUM read-read
pseudo-deps demoted to nosync), and a chain of single-wait NOPs on the
sync sequencer funnels every terminal before the kernel-tail drain.
"""

import numpy as np
import ml_dtypes

import concourse.bass as bass
import concourse.mybir as mybir
import concourse.tile as tile
from concourse.bass_utils import run_bass_kernel_spmd
from concourse.tile import add_dep_helper

N, M, P = 4096, 4096, 32
NCORES = 8
NSHARD = N // NCORES  # 512 rows of x1 per core
IBLK = 128            # output row-block = PSUM partition dim
JBLK = 512            # matmul free dim = one fp32 PSUM bank
PSW = 2048            # PSUM tile width (4 banks) = one exp-activation
NI = NSHARD // IBLK   # 4 row-blocks
NH = M // PSW         # 2 PSUM tiles per row-block
KST = 99              # stacked contraction: 32+32+32+3

BF16 = mybir.dt.bfloat16
NPBF16 = ml_dtypes.bfloat16

AW = NI * IBLK + 8    # A tile cols: 4 lhsT blocks + bias fp32 as bf16 pairs
XB_O = NI * IBLK


def _build_nc():
    nc = bass.Bass()
    a_t = nc.declare_dram_parameter("a_t", [IBLK, AW], BF16, isOutput=False)
    b0_t = nc.declare_dram_parameter("b0_t", [IBLK, PSW], BF16, isOutput=False)
    b1_t = nc.declare_dram_parameter("b1_t", [IBLK, PSW], BF16, isOutput=False)
    out = nc.declare_dram_parameter("out", [NSHARD, M], BF16, isOutput=True)

    with tile.TileContext(nc) as tc:
        with (
            tc.tile_pool(name="inp", bufs=1) as inp_pool,
            tc.tile_pool(name="stage", bufs=1) as stage_pool,
            tc.tile_pool(name="ps", bufs=1, space="PSUM") as ps_pool,
        ):
            dma_insts = []
            a_sb = inp_pool.tile([IBLK, AW], BF16, tag="a")
            dma_a = nc.scalar.dma_start(out=a_sb, in_=a_t[:, :])
            b_sb = inp_pool.tile([IBLK, M], BF16, tag="b")
            dma_b0 = nc.sync.dma_start(out=b_sb[:, 0:PSW], in_=b0_t[:, :])
            dma_b1 = nc.scalar.dma_start(out=b_sb[:, PSW:M], in_=b1_t[:, :])
            dma_insts += [dma_a, dma_b0, dma_b1]

            xbias = a_sb[:, XB_O : XB_O + 2 * NI].bitcast(mybir.dt.float32)

            # Warmup: tiny exp ACT so walrus's act-table load (~1.3us)
            # overlaps the input DMAs instead of gating the first real tile.
            scr = inp_pool.tile([IBLK, 1], mybir.dt.float32, tag="scr")
            nc.vector.memset(scr, 0.0)
            warm = inp_pool.tile([IBLK, 1], mybir.dt.float32, tag="warm")
            nc.scalar.activation(out=warm, in_=scr,
                                 func=mybir.ActivationFunctionType.Exp,
                                 bias=scr[:, 0:1], scale=1.0)

            # Real (Tile-tracked) tiny reads of the A tile so each engine
            # observes the A DMA semaphore once; later consumers on those
            # engines then carry only their single real wait.
            scr2 = inp_pool.tile([IBLK, 1], mybir.dt.float32, tag="scr2")
            nc.scalar.copy(out=scr2, in_=a_sb[:, 0:1])

            ps_tiles = [
                ps_pool.tile([IBLK, PSW], mybir.dt.float32, tag=f"ps{h}",
                             name=f"ps{h}")
                for h in range(NH)
            ]

            # Dummy K=1 matmul reading only the A tile: the tensor engine
            # observes the A DMA here; the first real matmul then carries
            # only the B0 wait.  Its PSUM write is overwritten by the real
            # start=True matmul into the same bank (same-engine FIFO order).
            nc.tensor.matmul(
                ps_tiles[0][:, 0:JBLK],
                lhsT=a_sb[0:1, 0:IBLK],
                rhs=a_sb[0:1, 0:JBLK],
                start=True,
                stop=True,
            )

            act_insts = []
            mm_insts = []
            for i in range(NI):
                out_sb = stage_pool.tile([IBLK, M], BF16, tag=f"out{i}",
                                         name=f"out{i}")
                lhsT = a_sb[0:KST, i * IBLK : (i + 1) * IBLK]
                for h in range(NH):
                    ps = ps_tiles[h]
                    for q in range(4):
                        mm_insts.append(
                            nc.tensor.matmul(
                                ps[:, q * JBLK : (q + 1) * JBLK],
                                lhsT=lhsT,
                                rhs=b_sb[0:KST, h * PSW + q * JBLK :
                                         h * PSW + (q + 1) * JBLK],
                                start=True,
                                stop=True,
                            )
                        )
                    act_insts.append(
                        nc.scalar.activation(
                            out=out_sb[:, h * PSW : (h + 1) * PSW],
                            in_=ps,
                            func=mybir.ActivationFunctionType.Exp,
                            bias=xbias[:, i : i + 1],
                            scale=1.0,
                        )
                    )
                    if i == NI - 1:
                        eng = nc.sync if h == 0 else nc.scalar
                        dma_insts.append(
                            eng.dma_start(
                                out=out[i * IBLK : (i + 1) * IBLK,
                                        h * PSW : (h + 1) * PSW],
                                in_=out_sb[:, h * PSW : (h + 1) * PSW],
                            )
                        )
                if i < NI - 1:
                    eng = nc.sync if i % 2 == 0 else nc.scalar
                    dma_insts.append(
                        eng.dma_start(
                            out=out[i * IBLK : (i + 1) * IBLK, :], in_=out_sb
                        )
                    )

            # Demote ACT->ACT pseudo-deps (PSUM bank read-read serialization,
            # already ordered through the interleaved matmuls + same-engine
            # FIFO) to nosync: walrus rejects multi-wait ACTIVATE.
            import bass_rust as _br

            act_names = {a.ins.name for a in act_insts}
            for a in act_insts:
                deps = list(a.ins.sync_dependency_names())
                spurious = [d for d in deps if d in act_names]
                if spurious:
                    keep = [d for d in deps if d not in act_names]
                    a.ins.take_sync_dependencies()
                    a.ins.set_sync_dependencies(
                        _br.InstructionNameOrderedSet(keep)
                    )
                    a.ins.add_nosync_dependencies_from(
                        _br.InstructionNameOrderedSet(spurious)
                    )

            # Wait-funnel so the framework's kernel-tail drain needs no waits
            # of its own (walrus rejects its usual all-sems wait list).
            for t in [mm_insts[-1], act_insts[-1], *dma_insts]:
                nop = nc.sync.nop(nofuse=True, hint="tail_funnel")
                add_dep_helper(nop.ins, t.ins, True, "tail wait funnel")
                for dd in dma_insts:
                    if dd is not t:
                        add_dep_helper(nop.ins, dd.ins, False, "funnel order")
    return nc


def _bf16_splits(x, n):
    """Split fp32 array into n bf16 parts summing to ~x."""
    parts = []
    rem = x.astype(np.float32)
    for _ in range(n):
        p = rem.astype(NPBF16)
        parts.append(p)
        rem = rem - p.astype(np.float32)
    return parts


def run(x1, x2, log_rho, log_sigma, trace=False):
    """Returns (K, exec_time_ns). exec_time_ns is None unless trace=True."""
    x1 = np.asarray(x1, dtype=np.float32)
    x2 = np.asarray(x2, dtype=np.float32)
    rho = float(np.exp(np.float64(np.asarray(log_rho))))
    log_sig = 2.0 * float(np.asarray(log_sigma))  # log(sigma)

    xs = (x1 / np.float32(rho)).astype(np.float32)
    ys = (x2 / np.float32(rho)).astype(np.float32)
    xn = np.einsum("np,np->n", xs, xs, dtype=np.float64)
    yn = np.einsum("mp,mp->m", ys, ys, dtype=np.float64)

    a = xs.T.astype(np.float32)  # (32, N)
    b = ys.T.astype(np.float32)  # (32, M)
    a_hi, a_lo = _bf16_splits(a, 2)
    b_hi, b_lo = _bf16_splits(b, 2)
    y1, y2, y3 = _bf16_splits((-0.5 * yn).astype(np.float32), 3)
    # per-row ACT bias: -0.5*||x_i||^2 + log(sigma), exact fp32
    xbias = ((-0.5 * xn) + log_sig).astype(np.float32)

    # B stack [99, 4096]: rows pair with lhsT rows [Ah; Ah; Al; ones3]
    bstack = np.zeros((IBLK, M), NPBF16)
    bstack[0:32] = b_hi
    bstack[32:64] = b_lo
    bstack[64:96] = b_hi
    bstack[96] = y1
    bstack[97] = y2
    bstack[98] = y3
    b0 = np.ascontiguousarray(bstack[:, 0:PSW])
    b1 = np.ascontiguousarray(bstack[:, PSW:M])

    nc = _build_nc()
    in_maps = []
    for c in range(NCORES):
        at = np.zeros((IBLK, AW), NPBF16)
        for i in range(NI):
            cols = slice(c * NSHARD + i * IBLK, c * NSHARD + (i + 1) * IBLK)
            blk = at[:, i * IBLK : (i + 1) * IBLK]
            blk[0:32] = a_hi[:, cols]
            blk[32:64] = a_hi[:, cols]
            blk[64:96] = a_lo[:, cols]
            blk[96:99] = NPBF16(1.0)
        xb = np.zeros((IBLK, NI), np.float32)
        for i in range(NI):
            xb[:, i] = xbias[c * NSHARD + i * IBLK : c * NSHARD + (i + 1) * IBLK]
        at[:, XB_O : XB_O + 2 * NI] = xb.view(np.uint16).view(NPBF16)
        in_maps.append(
            {"a_t": np.ascontiguousarray(at), "b0_t": b0, "b1_t": b1}
        )

    res = run_bass_kernel_spmd(
        nc, in_maps, core_ids=list(range(NCORES)), trace=trace
    )
    full = np.concatenate(
        [res.results[c]["out"] for c in range(NCORES)], axis=0
    ).astype(np.float32)
    return full, res.exec_time_ns


def kernel(x1, x2, log_rho, log_sigma):
    out, _ = run(x1, x2, log_rho, log_sigma, trace=False)
    return out


# revision 8
# speedup vs baseline: 1.1849x; 1.1849x over previous
"""Exponentiated-quadratic (RBF) kernel matrix on 8 Trainium2 NeuronCores.

K[i, j] = sigma * exp(-0.5 * ||x1_i/rho - x2_j/rho||^2)
        with sigma = exp(log_sigma)^2, rho = exp(log_rho)

Strategy (v3)
-------------
Row-shard x1 across the 8 cores (512 rows each), replicate x2.

Matmul: ONE fp8e4 DoubleRow matmul per 512-column PSUM bank computes the
entire contraction at 2 MACs/cycle/PE (512 cols in ~216ns @2.4GHz).  The
196 contraction slots (98 partition rows x 2 planes, planar k-subtile
layout via 3-dim [128,2,cols] tiles) stack six e4m3 split-product terms
of (4x)*(4y) -- A1B1, A1B2, A2B1, A1B3, A2B2, A3B1; dual-fp8 products
are exact in the e6m3/e10m10 datapath -- plus four power-of-2-weighted
rows carrying -0.5*||y_j||^2 * 16.  PSUM holds 16*(x.y - 0.5||y||^2);
measured end-to-end error 2.5e-3 rel-to-scale (bf16-output dominated).

Epilogue: one ScalarE exp-activation per [128,2048] PSUM tile:
exp(PSUM * 1/16 + bias) via ACT's free affine, with the per-row bias
(-0.5||x_i||^2 + 2 log_sigma) as the fp32 per-partition bias AP, writing
bf16 (host upcasts; bf16 rounding ~2e-3 << the 2e-2 gate, and it halves
the output-store HBM traffic).  A tiny warmup ACT at kernel start pulls
the ~1.3us exp table load off the critical path.

DMA: inputs fused so the first matmul waits on a single semaphore:
L0 = [lhsT stacks | bias | first output-column bank of B], then the rest
of B in two chunks; 3 loads + 5 stores = 8 HWDGE lanes, all dense
128-partition transfers.  Stores are issued per row-block as soon as its
second activation lands; the last block stores in halves.

walrus in this container rejects instructions carrying more than one
semaphore wait: tiny real reads make each engine observe the L0 DMA
once, activations carry only their PE wait (ACT->ACT PSUM read-read
pseudo-deps demoted to nosync), and a chain of single-wait NOPs on the
sync sequencer funnels every terminal before the kernel-tail drain.
"""

import numpy as np
import ml_dtypes

import concourse.bass as bass
import concourse.mybir as mybir
import concourse.tile as tile
from concourse.bass_utils import run_bass_kernel_spmd
from concourse.tile import add_dep_helper

N, M, P = 4096, 4096, 32
NCORES = 8
NSHARD = N // NCORES  # 512 rows of x1 per core
IBLK = 128            # output row-block = PSUM partition dim
JBLK = 512            # matmul free dim = one fp32 PSUM bank
PSW = 2048            # PSUM tile width (4 banks) = one exp-activation
NI = NSHARD // IBLK   # 4 row-blocks
NH = M // PSW         # 2 PSUM tiles per row-block
KP = 98               # partition rows used: 196 slots / 2 planes
SC = 4.0              # operand pre-scale; PSUM = 16 * S
YNV = (32.0, 2.0, 0.125, 2.0 ** -7)  # lhsT weights of the yn slot rows

FP8 = mybir.dt.float8e4
NPFP8 = ml_dtypes.float8_e4m3
BF16 = mybir.dt.bfloat16
NPBF16 = ml_dtypes.bfloat16

XB_O = NI * IBLK      # bias offset within a plane-0 row of L0
XB_W = 16             # 4 fp32 biases as 16 fp8 bytes
L0W = XB_O + XB_W + JBLK   # per-plane cols of L0: A stacks | bias | B bank 0
BRW = M - JBLK        # remaining output cols of B
BCH = BRW // 2        # per-DMA chunk (cols)

# term list: (A split idx, B split idx), 32 slots each; slots 192:196 = yn
TERMS = [(0, 0), (0, 1), (1, 0), (0, 2), (1, 1), (2, 0)]


def _slot(s):
    """slot index 0..195 -> (partition row, plane)."""
    return s % KP, s // KP


def _build_nc():
    nc = bass.Bass()
    l0_t = nc.declare_dram_parameter("l0_t", [IBLK, 2, L0W], FP8, isOutput=False)
    b1_t = nc.declare_dram_parameter("b1_t", [IBLK, 2, BCH], FP8, isOutput=False)
    b2_t = nc.declare_dram_parameter("b2_t", [IBLK, 2, BCH], FP8, isOutput=False)
    out = nc.declare_dram_parameter("out", [NSHARD, M], BF16, isOutput=True)

    with tile.TileContext(nc) as tc:
        with (
            tc.tile_pool(name="inp", bufs=1) as inp_pool,
            tc.tile_pool(name="stage", bufs=1) as stage_pool,
            tc.tile_pool(name="ps", bufs=1, space="PSUM") as ps_pool,
        ):
            dma_insts = []
            l0_sb = inp_pool.tile([IBLK, 2, L0W], FP8, tag="l0")
            dma_l0 = nc.sync.dma_start(out=l0_sb, in_=l0_t[:, :, :])
            b_sb = inp_pool.tile([IBLK, 2, BRW], FP8, tag="b")
            dma_b1 = nc.scalar.dma_start(
                out=b_sb[:, :, 0:BCH], in_=b1_t[:, :, :]
            )
            dma_b2 = nc.sync.dma_start(
                out=b_sb[:, :, BCH:BRW], in_=b2_t[:, :, :]
            )
            dma_insts += [dma_l0, dma_b1, dma_b2]

            def rhs_ap(h, q):
                """fp8 rhs [98, 2, 512] for PSUM bank q of tile h."""
                c = 2048 * h + 512 * q
                if c == 0:
                    return l0_sb[0:KP, :, XB_O + XB_W : XB_O + XB_W + JBLK]
                c -= JBLK
                return b_sb[0:KP, :, c : c + JBLK]

            xbias = l0_sb[:, 0:1, XB_O : XB_O + XB_W].bitcast(mybir.dt.float32)

            # Warmup: tiny exp ACT so walrus's act-table load (~1.3us)
            # overlaps the input DMAs instead of gating the first real tile.
            scr = inp_pool.tile([IBLK, 1], mybir.dt.float32, tag="scr")
            nc.vector.memset(scr, 0.0)
            warm = inp_pool.tile([IBLK, 1], mybir.dt.float32, tag="warm")
            nc.scalar.activation(out=warm, in_=scr,
                                 func=mybir.ActivationFunctionType.Exp,
                                 bias=scr[:, 0:1], scale=1.0)

            # Tiny real read of L0 so the scalar engine observes the L0 DMA
            # once; the activations then carry only their PE wait.
            scr2 = inp_pool.tile([IBLK, 1, 1], mybir.dt.float32, tag="scr2")
            nc.scalar.copy(out=scr2, in_=xbias[:, 0:1, 0:1])

            ps_tiles = [
                ps_pool.tile([IBLK, PSW], mybir.dt.float32, tag=f"ps{h}",
                             name=f"ps{h}")
                for h in range(NH)
            ]

            act_insts = []
            mm_insts = []
            for i in range(NI):
                out_sb = stage_pool.tile([IBLK, M], BF16, tag=f"out{i}",
                                         name=f"out{i}")
                lhsT = l0_sb[0:KP, :, i * IBLK : (i + 1) * IBLK]
                for h in range(NH):
                    ps = ps_tiles[h]
                    for q in range(4):
                        mm_insts.append(
                            nc.tensor.matmul(
                                ps[:, q * JBLK : (q + 1) * JBLK],
                                lhsT=lhsT,
                                rhs=rhs_ap(h, q),
                                start=True,
                                stop=True,
                                perf_mode=mybir.MatmulPerfMode.DoubleRow,
                            )
                        )
                    act_insts.append(
                        nc.scalar.activation(
                            out=out_sb[:, h * PSW : (h + 1) * PSW],
                            in_=ps,
                            func=mybir.ActivationFunctionType.Exp,
                            bias=xbias[:, 0:1, i : i + 1],
                            scale=1.0 / (SC * SC),
                        )
                    )
                    if i == NI - 1:
                        eng = nc.sync if h == 0 else nc.scalar
                        dma_insts.append(
                            eng.dma_start(
                                out=out[i * IBLK : (i + 1) * IBLK,
                                        h * PSW : (h + 1) * PSW],
                                in_=out_sb[:, h * PSW : (h + 1) * PSW],
                            )
                        )
                if i < NI - 1:
                    eng = nc.sync if i % 2 == 0 else nc.scalar
                    dma_insts.append(
                        eng.dma_start(
                            out=out[i * IBLK : (i + 1) * IBLK, :], in_=out_sb
                        )
                    )

            # Demote ACT->ACT pseudo-deps (PSUM bank read-read serialization,
            # already ordered through the interleaved matmuls + same-engine
            # FIFO) to nosync: walrus rejects multi-wait ACTIVATE.
            import bass_rust as _br

            act_names = {a.ins.name for a in act_insts}
            for a in act_insts:
                deps = list(a.ins.sync_dependency_names())
                spurious = [d for d in deps if d in act_names]
                if spurious:
                    keep = [d for d in deps if d not in act_names]
                    a.ins.take_sync_dependencies()
                    a.ins.set_sync_dependencies(
                        _br.InstructionNameOrderedSet(keep)
                    )
                    a.ins.add_nosync_dependencies_from(
                        _br.InstructionNameOrderedSet(spurious)
                    )

            # Wait-funnel so the framework's kernel-tail drain needs no waits
            # of its own (walrus rejects its usual all-sems wait list).
            for t in [mm_insts[-1], act_insts[-1], *dma_insts]:
                nop = nc.sync.nop(nofuse=True, hint="tail_funnel")
                add_dep_helper(nop.ins, t.ins, True, "tail wait funnel")
                for dd in dma_insts:
                    if dd is not t:
                        add_dep_helper(nop.ins, dd.ins, False, "funnel order")
    return nc


def _e4_splits(x, n):
    """Split fp32 array into n fp8e4m3 parts summing to ~x."""
    parts = []
    rem = x.astype(np.float32)
    for _ in range(n):
        p = np.clip(rem, -240, 240).astype(NPFP8)
        parts.append(p)
        rem = rem - p.astype(np.float32)
    return parts


def run(x1, x2, log_rho, log_sigma, trace=False):
    """Returns (K, exec_time_ns). exec_time_ns is None unless trace=True."""
    x1 = np.asarray(x1, dtype=np.float32)
    x2 = np.asarray(x2, dtype=np.float32)
    rho = float(np.exp(np.float64(np.asarray(log_rho))))
    log_sig = 2.0 * float(np.asarray(log_sigma))  # log(sigma)

    xs = (x1 / np.float32(rho)).astype(np.float32)
    ys = (x2 / np.float32(rho)).astype(np.float32)
    xn = np.einsum("np,np->n", xs, xs, dtype=np.float64)
    yn = np.einsum("mp,mp->m", ys, ys, dtype=np.float64)

    A = (xs.T * np.float32(SC)).astype(np.float32)  # (32, N)
    B = (ys.T * np.float32(SC)).astype(np.float32)  # (32, M)
    As = _e4_splits(A, 3)
    Bs = _e4_splits(B, 3)

    # yn slot rows: -0.5*yn*SC^2 decomposed over power-of-2 lhsT weights
    yrows = []
    rem = (-0.5 * yn * SC * SC).astype(np.float32)
    for v in YNV:
        r = np.clip(rem / np.float32(v), -240, 240).astype(NPFP8)
        yrows.append(r)
        rem = rem - np.float32(v) * r.astype(np.float32)

    # per-row ACT bias: -0.5*||x_i||^2 + log(sigma), exact fp32
    xbias = ((-0.5 * xn) + log_sig).astype(np.float32)

    # B stack [98, 2, M]: slot s -> (row s%98, plane s//98)
    bstack = np.zeros((IBLK, 2, M), NPFP8)
    for t, (ia, ib) in enumerate(TERMS):
        for d in range(32):
            k, p = (32 * t + d) % KP, (32 * t + d) // KP
            bstack[k, p] = Bs[ib][d]
    for j in range(4):
        k, p = (192 + j) % KP, (192 + j) // KP
        bstack[k, p] = yrows[j]

    b1 = np.ascontiguousarray(bstack[:, :, JBLK : JBLK + BCH])
    b2 = np.ascontiguousarray(bstack[:, :, JBLK + BCH :])

    nc = _build_nc()
    in_maps = []
    for c in range(NCORES):
        l0 = np.zeros((IBLK, 2, L0W), NPFP8)
        for i in range(NI):
            cols = slice(c * NSHARD + i * IBLK, c * NSHARD + (i + 1) * IBLK)
            for t, (ia, ib) in enumerate(TERMS):
                for d in range(32):
                    s = 32 * t + d
                    l0[s % KP, s // KP, i * IBLK : (i + 1) * IBLK] = As[ia][d, cols]
            for j in range(4):
                s = 192 + j
                l0[s % KP, s // KP, i * IBLK : (i + 1) * IBLK] = NPFP8(YNV[j])
        xb = np.zeros((IBLK, NI), np.float32)
        for i in range(NI):
            xb[:, i] = xbias[c * NSHARD + i * IBLK : c * NSHARD + (i + 1) * IBLK]
        l0[:, 0, XB_O : XB_O + XB_W] = xb.view(np.uint8).view(NPFP8)
        l0[:, :, XB_O + XB_W :] = bstack[:, :, 0:JBLK]
        in_maps.append(
            {"l0_t": np.ascontiguousarray(l0), "b1_t": b1, "b2_t": b2}
        )

    res = run_bass_kernel_spmd(
        nc, in_maps, core_ids=list(range(NCORES)), trace=trace
    )
    full = np.concatenate(
        [res.results[c]["out"] for c in range(NCORES)], axis=0
    ).astype(np.float32)
    return full, res.exec_time_ns


def kernel(x1, x2, log_rho, log_sigma):
    out, _ = run(x1, x2, log_rho, log_sigma, trace=False)
    return out
